# revision 9
# baseline (speedup 1.0000x reference)
"""BoundaryLoss (EDT-weighted BCE) on 8 Trainium2 NeuronCores — v2.

Layout: per core, partitions = 2 channels x 64 D-rows = 128; free dims =
(H=96, W=24 interior + 8 halo). 8 cores = 2 batches x 4 W-quarters.

Math: with a binary mask, every voxel has dist_pos = 0 or dist_neg = 0, so
w = ramp(dist_pos + dist_neg) = ramp(dist_pos) * ramp(dist_neg) with
ramp(a) = clamp(2.5 - 0.5a, 0, 1): the weight factorizes per channel and
the two EDT channels live in the two 64-partition blocks; the final
cross-channel product is one tensor_tensor with a quadrant-aligned
partition offset (no DMA).

Passes (separable squared EDT, +-4 window, distances capped at 5 which is
exact for w since any value >= 25 stays >= 25):
  1. W pass: two tensor_tensor_scans (fwd on DVE, bwd on Pool via reversed
     APs) compute the exact 1D distance-to-zero of f = 5*t / 5*(1-t); the
     scan chains across H rows but the 8 halo columns between consecutive
     rows make every cross-row leak path >= 5 steps, which the cap absorbs.
     min(fwd, bwd) on the 24 interior columns, then square.
  2. D pass: partitions shift by d=4..1 via DMA copies (2 per direction,
     one per channel block) of the pre-added tiles th_d = dW^2 + d^2;
     sentinel rows (100) pre-loaded from DRAM once per su/sd tile pair and
     preserved by descending-d reuse. Mins accumulate on DVE; the d=2/d=1
     pair-mins run on Pool.
  3. H pass: free-dim shifted mins (d=1..3 on DVE, d=4 pair-min on Pool).

BCE: q = t ? p : 1-p (copy_predicated), bce = min(-ln(q + 1e-44), 100)
with the 1e-44 bias reproducing torch's log clamp at -100 for q = 0.

Finalize per H-half: a = sqrt(g), u = clamp(2.5 - 0.5a, 0, 1) on all 128
partitions at once, m = u[0:64]*u[64:128], num/den accumulated per D-row by
Activation-engine accum_out; host reduces the 8 cores' [64, 4] partials in
float64 and applies the per-batch normalization + mean.
"""

import numpy as np

B, D, H, W = 2, 64, 96, 96
NQ = 4
WI = W // NQ      # 24 interior columns per core
HALO = 4
WE = WI + 2 * HALO  # 32
N_CORES = B * NQ
HH = H // 2       # finalize half

_CACHE = {}


def _build():
    import concourse.bacc as bacc
    import concourse.mybir as mybir
    import concourse.tile as tile

    fp32 = mybir.dt.float32
    bf16 = mybir.dt.bfloat16
    AF = mybir.ActivationFunctionType
    ALU = mybir.AluOpType

    nc = bacc.Bacc("TRN2", target_bir_lowering=False, debug=False)
    t_d = nc.dram_tensor("t", [128, H, WE], bf16, kind="ExternalInput").ap()
    p_d = nc.dram_tensor("p", [64, H, WI], fp32, kind="ExternalInput").ap()
    s_d = nc.dram_tensor("s", [4, 2, H, WI], bf16, kind="ExternalInput").ap()
    o_d = nc.dram_tensor("o", [64, 4], fp32, kind="ExternalOutput").ap()

    with tile.TileContext(nc) as tc:
        with tc.tile_pool(name="mem", bufs=1) as pool:
            t2 = pool.tile([128, H, WE], bf16)
            f = pool.tile([128, H, WE], bf16)
            sf = pool.tile([128, H, WE], bf16)
            sb = pool.tile([128, H, WE], bf16)
            m1 = pool.tile([128, H, WI], bf16)
            sq = pool.tile([128, H, WI], bf16)
            th = [pool.tile([128, H, WI], bf16, name=f"th{d}") for d in range(1, 5)]
            bsu = pool.tile([128, 2, H, WI], bf16)  # slot 0: d=4,2; slot 1: d=3,1
            bsd = pool.tile([128, 2, H, WI], bf16)
            gd = pool.tile([128, H, WI], bf16)
            gh = pool.tile([128, H, WI], bf16)
            pp = pool.tile([64, H, WI], fp32)
            q0 = pool.tile([64, H, WI], fp32)
            bce = pool.tile([64, H, WI], bf16)
            da = pool.tile([128, H, WI], bf16)
            uu = pool.tile([128, H, WI], bf16)
            mw = pool.tile([64, H, WI], bf16)
            u1t = pool.tile([64, H, WI], bf16)
            mb = pool.tile([64, H, WI], bf16)
            scr = pool.tile([64, H, WI], bf16)
            tm = pool.tile([64, H, WI], mybir.dt.uint8)
            s1 = pool.tile([128, 1], fp32)
            s2 = pool.tile([128, 1], fp32)
            eps = pool.tile([64, 1], fp32)
            onec = pool.tile([128, 1], bf16)
            os_ = pool.tile([64, 4], fp32)

            def tmin(out_ap, a_ap, b_ap, eng=None):
                (eng or nc.vector).tensor_tensor(out_ap, a_ap, b_ap, op=ALU.min)

            # channel-affine scalars: f = s1[p]*t + s2[p] -> 5t / 5-5t
            nc.vector.memset(s1[0:64], 5.0)
            nc.vector.memset(s1[64:128], -5.0)
            nc.vector.memset(s2[0:64], 0.0)
            nc.vector.memset(s2[64:128], 5.0)
            nc.vector.memset(onec[:], 1.0)
            nc.vector.memset(eps[:], 3.7835058e-44)

            # loads: t2 on SP queue; p + sentinels on the Act queue
            nc.sync.dma_start(t2[:], t_d)
            nc.scalar.dma_start(pp[:], p_d)
            nc.scalar.dma_start(bsu[60:64], s_d)
            nc.scalar.dma_start(bsu[124:128], s_d)
            nc.scalar.dma_start(bsd[0:4], s_d)
            nc.scalar.dma_start(bsd[64:68], s_d)

            # ---- W pass: scans along (h, w) ----
            nc.vector.tensor_scalar(
                f[:], t2[:], s1[:], s2[:], op0=ALU.mult, op1=ALU.add
            )
            ones = onec[:].broadcast_to((128, H * WE))
            f2 = f[:].opt()
            nc.vector.tensor_tensor_scan(
                sf[:].opt(), ones, f2, 100.0, op0=ALU.add, op1=ALU.min
            )
            nc.vector.tensor_tensor_scan(
                sb[:].opt()[:, ::-1], ones, f2[:, ::-1], 100.0,
                op0=ALU.add, op1=ALU.min,
            )
            lo, hi = HALO, HALO + WI
            tmin(m1[:], sf[:, :, lo:hi], sb[:, :, lo:hi])
            nc.scalar.activation(sq[:], m1[:], AF.Square)

            # ---- D pass: partition shifts, d descending ----
            nc.vector.tensor_scalar(th[3][:], sq[:], 1.0, 16.0,
                                    op0=ALU.mult, op1=ALU.add)
            nc.scalar.activation(th[2][:], sq[:], AF.Copy, bias=9.0)
            nc.scalar.activation(th[1][:], sq[:], AF.Copy, bias=4.0)
            nc.vector.tensor_scalar(th[0][:], sq[:], 1.0, 1.0,
                                    op0=ALU.mult, op1=ALU.add)
            for d, slot in ((4, 0), (3, 1), (2, 0), (1, 1)):
                thd = th[d - 1]
                nc.sync.dma_start(bsu[0:64 - d, slot], thd[d:64])
                nc.sync.dma_start(bsu[64:128 - d, slot], thd[64 + d:128])
                nc.sync.dma_start(bsd[d:64, slot], thd[0:64 - d])
                nc.sync.dma_start(bsd[64 + d:128, slot], thd[64:128 - d])
            tmin(gd[:], sq[:], bsu[:, 0])        # d=4 up (3-operand start)
            tmin(gd[:], bsd[:, 0], gd[:])        # d=4 down
            tmin(gd[:], bsu[:, 1], gd[:])        # d=3 up
            tmin(gd[:], bsd[:, 1], gd[:])        # d=3 down
            tmin(gd[:], bsu[:, 0], gd[:])        # d=2 up
            tmin(gd[:], bsd[:, 0], gd[:])        # d=2 down
            tmin(gd[:], bsu[:, 1], gd[:])        # d=1 up
            tmin(gd[:], bsd[:, 1], gd[:])        # d=1 down

            # ---- BCE (independent; fills DMA-wait gaps) ----
            nc.vector.tensor_scalar(q0[:], pp[:], -1.0, 1.0,
                                    op0=ALU.mult, op1=ALU.add)
            nc.gpsimd.tensor_copy(tm[:], t2[0:64, :, lo:hi])
            nc.vector.copy_predicated(q0[:], tm[:], pp[:])
            nc.scalar.activation(scr[:], q0[:], AF.Ln, bias=eps[:])
            nc.vector.tensor_scalar(bce[:], scr[:], -1.0, 100.0,
                                    op0=ALU.mult, op1=ALU.min)

            # ---- H pass: free-dim shifts ----
            nc.vector.tensor_scalar(th[0][:], gd[:], 1.0, 1.0,
                                    op0=ALU.mult, op1=ALU.add)
            nc.vector.tensor_scalar(th[3][:], gd[:], 1.0, 16.0,
                                    op0=ALU.mult, op1=ALU.add)
            nc.scalar.activation(th[1][:], gd[:], AF.Copy, bias=4.0)
            nc.scalar.activation(th[2][:], gd[:], AF.Copy, bias=9.0)
            nc.vector.tensor_copy(gh[:, 0:1, :], gd[:, 0:1, :])
            tmin(gh[:, 1:96], gd[:, 1:96], th[0][:, 0:95])
            tmin(gh[:, 0:95], th[0][:, 1:96], gh[:, 0:95])
            tmin(gh[:, 2:96], th[1][:, 0:94], gh[:, 2:96])
            tmin(gh[:, 0:94], th[1][:, 2:96], gh[:, 0:94])
            tmin(gh[:, 3:96], th[2][:, 0:93], gh[:, 3:96])
            tmin(gh[:, 0:93], th[2][:, 3:96], gh[:, 0:93])
            tmin(gh[:, 4:96], th[3][:, 0:92], gh[:, 4:96])
            tmin(gh[:, 0:92], th[3][:, 4:96], gh[:, 0:92])

            # ---- finalize, per H-half ----
            for i, (a, b) in enumerate(((0, HH), (HH, H))):
                hs = np.s_[:, a:b, :]
                nc.scalar.activation(da[hs], gh[hs], AF.Sqrt)
                nc.vector.tensor_scalar(uu[hs], da[hs], -0.5, 2.5,
                                        op0=ALU.mult, op1=ALU.add)
                nc.vector.tensor_scalar(uu[hs], uu[hs], 1.0, 0.0,
                                        op0=ALU.min, op1=ALU.max)
                nc.vector.tensor_scalar(u1t[:, a:b, :], uu[64:128, a:b, :],
                                        1.0, 0.0, op0=ALU.mult, op1=ALU.add)
                nc.vector.tensor_tensor(
                    mw[:, a:b, :], uu[0:64, a:b, :], u1t[:, a:b, :],
                    op=ALU.mult,
                )
                nc.scalar.activation(scr[:, a:b, :], mw[:, a:b, :], AF.Copy,
                                     accum_out=os_[:, 2 + i:3 + i])
                nc.vector.tensor_tensor(
                    mb[:, a:b, :], mw[:, a:b, :], bce[:, a:b, :], op=ALU.mult
                )
                nc.scalar.activation(scr[:, a:b, :], mb[:, a:b, :], AF.Copy,
                                     accum_out=os_[:, i:i + 1])
            nc.sync.dma_start(o_d, os_[:])
    nc.compile()
    return nc


def _get_nc():
    if "nc" not in _CACHE:
        _CACHE["nc"] = _build()
    return _CACHE["nc"]


def _slabs(pred, target):
    import ml_dtypes

    bf16 = ml_dtypes.bfloat16
    tp = np.pad(
        np.asarray(target, dtype=np.float32),
        ((0, 0), (0, 0), (0, 0), (HALO, HALO)),
        mode="edge",
    )  # [B, D, H, W+8]
    pr = np.asarray(pred, dtype=np.float32)
    sent = np.full((4, 2, H, WI), 100.0, dtype=bf16)
    in_maps = []
    for b in range(B):
        for q in range(NQ):
            ts_ = np.ascontiguousarray(
                tp[b, :, :, q * WI: q * WI + WE]
            ).astype(bf16)  # [64, H, WE]
            t2 = np.concatenate([ts_, ts_], axis=0)  # [128, H, WE]
            ps = np.ascontiguousarray(pr[b, :, :, q * WI:(q + 1) * WI])
            in_maps.append({"t": t2, "p": ps, "s": sent})
    return in_maps


def kernel(pred: np.ndarray, target: np.ndarray) -> np.ndarray:
    from concourse.bass_utils import run_bass_kernel_spmd

    nc = _get_nc()
    in_maps = _slabs(pred, target)
    res = run_bass_kernel_spmd(nc, in_maps, list(range(N_CORES)))

    loss = 0.0
    for b in range(B):
        num = 0.0
        den = 0.0
        for q in range(NQ):
            o = res.results[b * NQ + q]["o"].astype(np.float64)
            num += o[:, 0].sum() + o[:, 1].sum()
            den += o[:, 2].sum() + o[:, 3].sum()
        loss += num / (den + 1e-5)
    return np.float32(loss / B)


# revision 11
# speedup vs baseline: 1.1024x; 1.1024x over previous
"""BoundaryLoss (EDT-weighted BCE) on 8 Trainium2 NeuronCores — v2.

Layout: per core, partitions = 2 channels x 64 D-rows = 128; free dims =
(H=96, W=24 interior + 8 halo). 8 cores = 2 batches x 4 W-quarters.

Math: with a binary mask, every voxel has dist_pos = 0 or dist_neg = 0, so
w = ramp(dist_pos + dist_neg) = ramp(dist_pos) * ramp(dist_neg) with
ramp(a) = clamp(2.5 - 0.5a, 0, 1): the weight factorizes per channel and
the two EDT channels live in the two 64-partition blocks; the final
cross-channel product is one tensor_tensor with a quadrant-aligned
partition offset (no DMA).

Passes (separable squared EDT, +-4 window, distances capped at 5 which is
exact for w since any value >= 25 stays >= 25):
  1. W pass: two tensor_tensor_scans (fwd on DVE, bwd on Pool via reversed
     APs) compute the exact 1D distance-to-zero of f = 5*t / 5*(1-t); the
     scan chains across H rows but the 8 halo columns between consecutive
     rows make every cross-row leak path >= 5 steps, which the cap absorbs.
     min(fwd, bwd) on the 24 interior columns, then square.
  2. D pass: partitions shift by d=4..1 via DMA copies (2 per direction,
     one per channel block) of the pre-added tiles th_d = dW^2 + d^2;
     sentinel rows (100) pre-loaded from DRAM once per su/sd tile pair and
     preserved by descending-d reuse. Mins accumulate on DVE; the d=2/d=1
     pair-mins run on Pool.
  3. H pass: free-dim shifted mins (d=1..3 on DVE, d=4 pair-min on Pool).

BCE: q = t ? p : 1-p (copy_predicated), bce = min(-ln(q + 1e-44), 100)
with the 1e-44 bias reproducing torch's log clamp at -100 for q = 0.

Finalize per H-half: a = sqrt(g), u = clamp(2.5 - 0.5a, 0, 1) on all 128
partitions at once, m = u[0:64]*u[64:128], num/den accumulated per D-row by
Activation-engine accum_out; host reduces the 8 cores' [64, 4] partials in
float64 and applies the per-batch normalization + mean.
"""

import numpy as np

B, D, H, W = 2, 64, 96, 96
NQ = 4
WI = W // NQ      # 24 interior columns per core
HALO = 4
WE = WI + 2 * HALO  # 32
N_CORES = B * NQ
HH = H // 2       # finalize half

_CACHE = {}


def _build():
    import concourse.bacc as bacc
    import concourse.mybir as mybir
    import concourse.tile as tile

    fp32 = mybir.dt.float32
    bf16 = mybir.dt.bfloat16
    AF = mybir.ActivationFunctionType
    ALU = mybir.AluOpType

    nc = bacc.Bacc("TRN2", target_bir_lowering=False, debug=False)
    t_d = nc.dram_tensor("t", [128, H, WE], bf16, kind="ExternalInput").ap()
    p_d = nc.dram_tensor("p", [64, H, WI], fp32, kind="ExternalInput").ap()
    s_d = nc.dram_tensor("s", [4, 4, H, WI], bf16, kind="ExternalInput").ap()
    o_d = nc.dram_tensor("o", [64, 4], fp32, kind="ExternalOutput").ap()

    with tile.TileContext(nc) as tc:
        with tc.tile_pool(name="mem", bufs=1) as pool:
            t2 = pool.tile([128, H, WE], bf16)
            f = pool.tile([128, H, WE], bf16)
            sf = pool.tile([128, H, WE], bf16)
            sb = pool.tile([128, H, WE], bf16)
            m1 = pool.tile([128, H, WI], bf16)
            sq = pool.tile([128, H, WI], bf16)
            th = [pool.tile([128, H, WI], bf16, name=f"th{d}") for d in range(1, 5)]
            bsu = pool.tile([128, 4, H, WI], bf16)  # slot d-1 per direction
            bsd = pool.tile([128, 4, H, WI], bf16)
            gd = pool.tile([128, H, WI], bf16)
            gh = pool.tile([128, H, WI], bf16)
            pp = pool.tile([64, H, WI], fp32)
            q0 = pool.tile([64, H, WI], fp32)
            bce = pool.tile([64, H, WI], bf16)
            da = pool.tile([128, H, WI], bf16)
            uu = pool.tile([128, H, WI], bf16)
            mw = pool.tile([64, H, WI], bf16)
            u1t = pool.tile([64, H, WI], bf16)
            mb = pool.tile([64, H, WI], bf16)
            scr = pool.tile([64, H, WI], bf16)
            tm = pool.tile([64, H, WI], mybir.dt.uint8)
            s1 = pool.tile([128, 1], fp32)
            s2 = pool.tile([128, 1], fp32)
            eps = pool.tile([64, 1], fp32)
            onec = pool.tile([128, 1], bf16)
            os_ = pool.tile([64, 4], fp32)

            def tmin(out_ap, a_ap, b_ap, eng=None):
                (eng or nc.vector).tensor_tensor(out_ap, a_ap, b_ap, op=ALU.min)

            # channel-affine scalars: f = s1[p]*t + s2[p] -> 5t / 5-5t
            nc.vector.memset(s1[0:64], 5.0)
            nc.vector.memset(s1[64:128], -5.0)
            nc.vector.memset(s2[0:64], 0.0)
            nc.vector.memset(s2[64:128], 5.0)
            nc.vector.memset(onec[:], 1.0)
            nc.vector.memset(eps[:], 3.7835058e-44)

            # loads: t2 on SP queue; p + sentinels on the Act queue
            nc.sync.dma_start(t2[:], t_d)
            nc.scalar.dma_start(pp[:], p_d)
            nc.scalar.dma_start(bsu[60:64], s_d)
            nc.scalar.dma_start(bsu[124:128], s_d)
            nc.scalar.dma_start(bsd[0:4], s_d)
            nc.scalar.dma_start(bsd[64:68], s_d)
            nc.scalar.activation(os_[:, 0:1], eps[:], AF.Ln)  # pin natural_log table

            # ---- W pass: scans along (h, w) ----
            nc.vector.tensor_scalar(
                f[:], t2[:], s1[:], s2[:], op0=ALU.mult, op1=ALU.add
            )
            ones = onec[:].broadcast_to((128, H * WE))
            f2 = f[:].opt()
            nc.vector.tensor_tensor_scan(
                sf[:].opt(), ones, f2, 100.0, op0=ALU.add, op1=ALU.min
            )
            nc.vector.tensor_tensor_scan(
                sb[:].opt()[:, ::-1], ones, f2[:, ::-1], 100.0,
                op0=ALU.add, op1=ALU.min,
            )
            lo, hi = HALO, HALO + WI
            tmin(m1[:], sf[:, :, lo:hi], sb[:, :, lo:hi])
            nc.scalar.activation(sq[:], m1[:], AF.Square)

            # ---- D pass: partition shifts, d descending ----
            nc.vector.tensor_scalar(th[3][:], sq[:], 1.0, 16.0,
                                    op0=ALU.mult, op1=ALU.add)
            nc.scalar.activation(th[2][:], sq[:], AF.Copy, bias=9.0)
            nc.scalar.activation(th[1][:], sq[:], AF.Copy, bias=4.0)
            nc.vector.tensor_scalar(th[0][:], sq[:], 1.0, 1.0,
                                    op0=ALU.mult, op1=ALU.add)
            for d in (4, 3, 2, 1):
                slot = d - 1
                thd = th[d - 1]
                nc.sync.dma_start(bsu[0:64 - d, slot], thd[d:64])
                nc.sync.dma_start(bsu[64:128 - d, slot], thd[64 + d:128])
                nc.sync.dma_start(bsd[d:64, slot], thd[0:64 - d])
                nc.sync.dma_start(bsd[64 + d:128, slot], thd[64:128 - d])
                if d == 4:
                    tmin(gd[:], sq[:], bsu[:, slot])   # 3-operand start
                else:
                    tmin(gd[:], bsu[:, slot], gd[:])
                tmin(gd[:], bsd[:, slot], gd[:])

            # ---- BCE (independent; scheduled into the D-phase DMA window) ----
            nc.gpsimd.tensor_copy(tm[:], t2[0:64, :, lo:hi])
            with tc.tile_wait_until(0.0175):
                nc.vector.tensor_scalar(q0[:], pp[:], -1.0, 1.0,
                                        op0=ALU.mult, op1=ALU.add)
                nc.vector.copy_predicated(q0[:], tm[:], pp[:])
            nc.scalar.activation(scr[:], q0[:], AF.Ln, bias=eps[:])
            with tc.tile_wait_until(0.024):
                nc.vector.tensor_scalar(bce[:], scr[:], -1.0, 100.0,
                                        op0=ALU.mult, op1=ALU.min)

            # ---- H pass: free-dim shifts ----
            nc.vector.tensor_scalar(th[0][:], gd[:], 1.0, 1.0,
                                    op0=ALU.mult, op1=ALU.add)
            nc.vector.tensor_scalar(th[3][:], gd[:], 1.0, 16.0,
                                    op0=ALU.mult, op1=ALU.add)
            nc.scalar.activation(th[1][:], gd[:], AF.Copy, bias=4.0)
            nc.scalar.activation(th[2][:], gd[:], AF.Copy, bias=9.0)
            nc.vector.tensor_copy(gh[:, 0:1, :], gd[:, 0:1, :])
            tmin(gh[:, 1:96], gd[:, 1:96], th[0][:, 0:95])
            tmin(gh[:, 0:95], th[0][:, 1:96], gh[:, 0:95])
            tmin(gh[:, 2:96], th[1][:, 0:94], gh[:, 2:96])
            tmin(gh[:, 0:94], th[1][:, 2:96], gh[:, 0:94])
            tmin(gh[:, 3:96], th[2][:, 0:93], gh[:, 3:96])
            tmin(gh[:, 0:93], th[2][:, 3:96], gh[:, 0:93])
            tmin(gh[:, 4:96], th[3][:, 0:92], gh[:, 4:96])
            tmin(gh[:, 0:92], th[3][:, 4:96], gh[:, 0:92])

            # ---- finalize, per H-half ----
            for i, (a, b) in enumerate(((0, HH), (HH, H))):
                hs = np.s_[:, a:b, :]
                nc.scalar.activation(da[hs], gh[hs], AF.Sqrt)
                nc.vector.tensor_scalar(uu[hs], da[hs], -0.5, 2.5,
                                        op0=ALU.mult, op1=ALU.add)
                nc.vector.tensor_scalar(uu[hs], uu[hs], 1.0, 0.0,
                                        op0=ALU.min, op1=ALU.max)
                nc.vector.tensor_scalar(u1t[:, a:b, :], uu[64:128, a:b, :],
                                        1.0, 0.0, op0=ALU.mult, op1=ALU.add)
                nc.vector.tensor_tensor(
                    mw[:, a:b, :], uu[0:64, a:b, :], u1t[:, a:b, :],
                    op=ALU.mult,
                )
                nc.scalar.activation(scr[:, a:b, :], mw[:, a:b, :], AF.Copy,
                                     accum_out=os_[:, 2 * i + 1:2 * i + 2])
                nc.vector.tensor_tensor(
                    mb[:, a:b, :], mw[:, a:b, :], bce[:, a:b, :], op=ALU.mult
                )
                nc.scalar.activation(scr[:, a:b, :], mb[:, a:b, :], AF.Copy,
                                     accum_out=os_[:, 2 * i:2 * i + 1])
                nc.sync.dma_start(o_d[:, 2 * i:2 * i + 2], os_[:, 2 * i:2 * i + 2])
    nc.compile()
    return nc


def _get_nc():
    if "nc" not in _CACHE:
        _CACHE["nc"] = _build()
    return _CACHE["nc"]


def _slabs(pred, target):
    import ml_dtypes

    bf16 = ml_dtypes.bfloat16
    tp = np.pad(
        np.asarray(target, dtype=np.float32),
        ((0, 0), (0, 0), (0, 0), (HALO, HALO)),
        mode="edge",
    )  # [B, D, H, W+8]
    pr = np.asarray(pred, dtype=np.float32)
    sent = np.full((4, 4, H, WI), 100.0, dtype=bf16)
    in_maps = []
    for b in range(B):
        for q in range(NQ):
            ts_ = np.ascontiguousarray(
                tp[b, :, :, q * WI: q * WI + WE]
            ).astype(bf16)  # [64, H, WE]
            t2 = np.concatenate([ts_, ts_], axis=0)  # [128, H, WE]
            ps = np.ascontiguousarray(pr[b, :, :, q * WI:(q + 1) * WI])
            in_maps.append({"t": t2, "p": ps, "s": sent})
    return in_maps


def kernel(pred: np.ndarray, target: np.ndarray) -> np.ndarray:
    from concourse.bass_utils import run_bass_kernel_spmd

    nc = _get_nc()
    in_maps = _slabs(pred, target)
    res = run_bass_kernel_spmd(nc, in_maps, list(range(N_CORES)))

    loss = 0.0
    for b in range(B):
        num = 0.0
        den = 0.0
        for q in range(NQ):
            o = res.results[b * NQ + q]["o"].astype(np.float64)
            num += o[:, 0].sum() + o[:, 2].sum()
            den += o[:, 1].sum() + o[:, 3].sum()
        loss += num / (den + 1e-5)
    return np.float32(loss / B)
